# revision 39
# baseline (speedup 1.0000x reference)
"""Fused LayerNorm + causal multi-head attention (with additive bias) + out-proj
for Trainium2, SPMD over 8 NeuronCores.

Sharding: tensor-parallel over heads. 16 heads / 8 cores = 2 heads per core.
Each core computes LN(x) (replicated), the qkv projection restricted to its
2 heads' columns, causal softmax attention with its heads' bias slices, and a
partial output projection (its heads' rows of w_out). Host sums the 8 partial
outputs (the TP all-reduce, done on gather) in f32 from bf16 partials.

Key choices (engine queues are strict FIFO; emission order is the schedule):
 - x arrives pre-transposed ([dim, token], bf16). The LN mean-subtraction is
   folded into the weights on the HOST: W'' = gW - (1/D) 1 colsum(gW), so
   x^T W'' = (x - mu)^T gW directly. Only rsig (1/std) is applied on device,
   riding the PSUM->SBUF eviction (DVE multiply against a broadcast f32 rsig
   row). The beta term is a rank-1 PSUM matmul (skipped when beta == 0).
 - Input DMAs are emitted in consumption order; x streams through a 3-buffer
   rotating pool of [128, KS, 512] blocks whose refill DMAs are emitted right
   after each block's last reader.
 - PE tile-level concurrency: matmuls whose array footprints are disjoint
   (different 32-row/col groups) execute concurrently. The two heads' C=64
   score matmuls are emitted adjacently -> row groups (0,0)/(64,0) overlap.
   The qkv and out-proj matmuls are emitted as column-split M=64 half pairs
   (tiles (0,0)/(0,64) share the moving operand via two XBUSes) -> 2x.
 - x^2 for the LN variance is computed on the DVE (bf16 tensor-tensor mult
   at 2x rate) to keep the ScalarE queue clear for the short mu^2/sqrt row
   ops on the rsig critical chain.
 - LN stats live in ONE PSUM bank: mean row at partition 0, sum(x^2) row at
   partition 32; the two ones-vector accumulation chains interleave.
 - The causal mask and softmax max-subtraction are folded into the host bias
   (pre-masked with -1e9; logits are O(10) so exp never overflows).
 - The attention bias for each i-tile column is cached in SBUF via one large
   DMA per (head, i-tile) - 8 transfers replace 80 small ones.
 - Batch is OUTER within each i-tile so only one batch's score tiles rotate
   through the 2-buffer score PSUM pool and one batch's P@V accumulators are
   live. Score PSUM tiles span two banks ([128, 2, 512]) so one exp ACTIVATE
   covers two j-tiles; P@V matmuls are emitted two units late so the PE
   never stalls on an exp.
 - Softmax normalization is deferred: an all-ones column appended to V gives
   the row sums l_i for free; the P@V accumulators are evicted to SBUF by
   the ScalarE (fast PSUM free for the next unit), and 1/l is applied from
   SBUF on the DVE/GpSimd. Each unit's normalization + projection trickles
   between the NEXT unit's groups; the projection owns dedicated PSUM banks
   and its evictions alternate ScalarE/VectorE.
"""

import numpy as np
import ml_dtypes
from contextlib import ExitStack

import concourse.bass as bass
import concourse.tile as tile
from concourse import bacc, mybir
from concourse.bass_utils import run_bass_kernel_spmd

F32 = mybir.dt.float32
BF16 = mybir.dt.bfloat16
AL = mybir.AluOpType
AF = mybir.ActivationFunctionType

N_CORES = 8
B = 2            # batch
N = 2048         # tokens
D = 1024         # model dim
H = 16           # total heads
HL = 2           # heads per core
DH = 64          # head dim
COLS = 3 * HL * DH   # 384 qkv columns per core
KS = D // 128    # 8 contraction slabs
TT = N // 128    # 16 token tiles
IT = N // 512    # 4 i-tiles (query tiles of 512)
SCALE = DH ** -0.5
LN_EPS = 1e-5
NEG = -1.0e9


def build_program(bw_zero=True, debug=False):
    nc = bacc.Bacc("TRN2", target_bir_lowering=False, debug=False)

    xT_in = nc.dram_tensor("xT", [B, D, N], BF16, kind="ExternalInput")
    biasT_in = nc.dram_tensor("biasT", [HL, N, N], BF16, kind="ExternalInput")
    wqkv_in = nc.dram_tensor("wqkv", [D, COLS], BF16, kind="ExternalInput")
    wout_in = nc.dram_tensor("wout", [HL * DH, D], BF16, kind="ExternalInput")
    bw_in = nc.dram_tensor("bw", [1, COLS], BF16, kind="ExternalInput")
    ident_in = nc.dram_tensor("ident", [128, 128], BF16, kind="ExternalInput")
    y_out = nc.dram_tensor("y", [B, N, D], BF16, kind="ExternalOutput")

    MM = dict(skip_group_check=True)

    with tile.TileContext(nc) as tc, ExitStack() as ctx:
        # ---- persistent sbuf ----
        pers = ctx.enter_context(tc.tile_pool(name="pers", bufs=1))
        qT = [pers.tile([128, N], BF16, tag=f"qT{b}", name=f"qT{b}") for b in range(B)]
        kT = [pers.tile([128, N], BF16, tag=f"kT{b}", name=f"kT{b}") for b in range(B)]
        vT = [pers.tile([128, N], BF16, tag=f"vT{b}", name=f"vT{b}") for b in range(B)]
        # V natural with ones column: per key-tile [.., 130]: h0 v(64)+1, h1 v(64)+1
        vA = [pers.tile([128, TT, 130], BF16, tag=f"vA{b}", name=f"vA{b}") for b in range(B)]
        oT = [pers.tile([128, N], BF16, tag=f"oT{b}", name=f"oT{b}") for b in range(B)]
        ident = pers.tile([128, 128], BF16, tag="ident")
        wqb = pers.tile([128, KS, COLS], BF16, tag="wqb")
        wob = pers.tile([128, D], BF16, tag="wob")
        onesd = pers.tile([128, 1], BF16, tag="onesd")    # 1/D for stats matmuls
        epsc = pers.tile([128, 1], F32, tag="epsc")

        # attention bias SBUF cache: one i-tile column of all j-tiles, both
        # heads, double-buffered -> 8 large DMAs replace 80 small ones
        bias_pool = ctx.enter_context(tc.tile_pool(name="bias", bufs=2))
        bias_tiles = {}

        def load_bias(t):
            bc = bias_pool.tile([128, HL, TT, 512], BF16, tag="bc",
                                name=f"bc{t}")
            nj = 4 * (t + 1)

            def src(h):
                return (biasT_in.ap()[h, 0:128 * nj, t * 512:(t + 1) * 512]
                        .rearrange("(j p) i -> p j i", p=128))

            nc.sync.dma_start(bc[:, 0, 0:nj, :], src(0))
            # head 1 rotated by 64 partitions: row p of each j-tile lands at
            # partition (p+64)%128, so its identity-add uses the off-diagonal
            # PE tiles (0,64)/(64,0) - cell-disjoint from head 0's diagonal
            # halves, letting both heads' bias adds share one PE slot
            nc.sync.dma_start(bc[64:128, 1, 0:nj, :], src(1)[0:64])
            nc.sync.dma_start(bc[0:64, 1, 0:nj, :], src(1)[64:128])
            bias_tiles[t] = bc

        # ---- input DMAs in consumption order; x streams through a 3-buffer
        # rotating pool (one [128, KS, 512] block per (b, nt)) ----
        xpool = ctx.enter_context(tc.tile_pool(name="xT", bufs=3))

        def xb_tile(b, nt):
            xb = xpool.tile([128, KS, 512], BF16, tag="xb", name=f"xb{b}{nt}")
            sl = slice(nt * 512, (nt + 1) * 512)
            nc.sync.dma_start(
                xb[:], xT_in.ap()[b, :, sl].rearrange("(k p) n -> p k n", p=128))
            return xb

        blocks = [(b, nt) for b in range(B) for nt in range(IT)]
        xts = {}
        xts[blocks[0]] = xb_tile(*blocks[0])
        nc.sync.dma_start(
            wqb[:], wqkv_in.ap().rearrange("(k p) c -> p k c", p=128))
        for blk in blocks[1:3]:
            xts[blk] = xb_tile(*blk)
        load_bias(0)
        nc.sync.dma_start(ident[:], ident_in.ap())
        nc.sync.dma_start(wob[:], wout_in.ap())
        load_bias(1)
        nc.vector.memset(onesd[:], 1.0 / D)
        nc.vector.memset(epsc[:], LN_EPS)
        for b in range(B):
            nc.vector.memset(
                vA[b][:, :, 64::65].rearrange("p t o -> p (t o)"), 1.0)
        if not bw_zero:
            bwb = pers.tile([1, COLS], BF16, tag="bwb")
            nc.sync.dma_start(bwb[:], bw_in.ap())

        # ---- LN stats + qkv^T, per batch; stage-pipelined emission ----
        tree = ctx.enter_context(tc.tile_pool(name="tree", bufs=2))
        rows = ctx.enter_context(tc.tile_pool(name="rows", bufs=2))
        rbc = ctx.enter_context(tc.tile_pool(name="rbc", bufs=2))
        x2p = ctx.enter_context(tc.tile_pool(name="x2p", bufs=2))
        with tc.tile_pool(name="pstat", bufs=1, space="PSUM") as pstat, \
             tc.tile_pool(name="pqkv", bufs=3, space="PSUM") as pqkv, \
             tc.tile_pool(name="pvt", bufs=2, space="PSUM") as pvt:
            dsts = (qT, kT, vT)

            def stage_a(b, nt):
                """x^2 squares (DVE mult) + x slab pre-sum tree (DVE only)."""
                xb = xts[(b, nt)]
                x2t = x2p.tile([128, KS, 512], BF16, tag="x2", name=f"x2_{b}_{nt}")
                nc.vector.tensor_tensor(x2t[:], xb[:], xb[:], op=AL.mult)
                a1 = tree.tile([128, 4, 512], BF16, tag="a1", name=f"a1_{b}_{nt}")
                c1 = tree.tile([128, 2, 512], BF16, tag="c1", name=f"c1_{b}_{nt}")
                nc.vector.tensor_tensor(a1[:], xb[:, 0::2, :], xb[:, 1::2, :],
                                        op=AL.add)
                nc.vector.tensor_tensor(c1[:], a1[:, 0::2, :], a1[:, 1::2, :],
                                        op=AL.add)
                return (x2t, c1)

            def stage_b(b, nt, x2t, c1):
                """stat matmuls: mean row at partition 0, sum(x^2) row at
                partition 32 of ONE psum bank; the two ones-vector chains
                interleave (disjoint col strips)."""
                stat = pstat.tile([128, 512], F32, tag="stat", name="stat")
                mu_ps = stat[0:1, :]
                x2_ps = stat[32:33, :]
                # the has_written clear (start=True) applies to the zero
                # region of the *written* partitions only, so the two chains
                # each open their own group in the shared bank
                nc.tensor.matmul(mu_ps, onesd[:], c1[:, 0, :],
                                 start=True, stop=False, **MM)
                nc.tensor.matmul(x2_ps, onesd[:], x2t[:, 0, :],
                                 start=True, stop=False, **MM)
                nc.tensor.matmul(mu_ps, onesd[:], c1[:, 1, :],
                                 start=False, stop=True, **MM)
                for k in range(1, KS):
                    nc.tensor.matmul(x2_ps, onesd[:], x2t[:, k, :],
                                     start=False, stop=(k == KS - 1), **MM)
                mu2_r = rows.tile([1, 512], F32, tag="mu2_r", bufs=1)
                nc.scalar.activation(mu2_r[:], mu_ps, AF.Square)
                var_r = rows.tile([1, 512], F32, tag="var_r", bufs=1)
                nc.vector.tensor_tensor(var_r[:], x2_ps, mu2_r[:],
                                        op=AL.subtract)
                sd_r = rows.tile([1, 512], F32, tag="sd_r")
                nc.scalar.activation(sd_r[:], var_r[:], AF.Sqrt, bias=epsc[0:1, :])
                rsig_r = rows.tile([1, 512], F32, tag="rsig_r")
                nc.vector.reciprocal_approx_fast(rsig_r[:], sd_r[:])
                rsig_bc = rbc.tile([128, 512], F32, tag="rsbc")
                nc.gpsimd.partition_broadcast(rsig_bc[:], rsig_r[:], channels=128)
                sd_bf = None
                if not bw_zero:
                    sd_bf = rows.tile([1, 512], BF16, tag="sd_bf")
                    nc.vector.tensor_copy(sd_bf[:], sd_r[:])
                return rsig_bc, sd_bf

            def stage_c(b, nt, rsig_bc, sd_bf):
                """qkv matmuls (column-split M=64 concurrent half pairs) +
                rsig eviction to qT/kT/vT."""
                sl = slice(nt * 512, (nt + 1) * 512)
                xb = xts[(b, nt)]
                for blk in range(3):
                    ps = pqkv.tile([128, 512], F32, tag="psqkv")
                    for k in range(KS):
                        for hf in range(2):
                            cs = slice(blk * 128 + 64 * hf,
                                       blk * 128 + 64 * (hf + 1))
                            nc.tensor.matmul(
                                ps[64 * hf:64 * (hf + 1), :],
                                wqb[:, k, cs], xb[:, k, :],
                                start=(k == 0),
                                stop=(k == KS - 1 and bw_zero), **MM)
                    if not bw_zero:
                        csl = slice(blk * 128, (blk + 1) * 128)
                        nc.tensor.matmul(ps[:], bwb[0:1, csl], sd_bf[0:1, :],
                                         start=False, stop=True, **MM)
                    nc.vector.tensor_tensor(dsts[blk][b][:, sl], ps[:],
                                            rsig_bc[:], op=AL.mult)

            def stage_d(b, nt):
                """V -> natural layout: 4 PE transposes + one ScalarE copy."""
                pst = pvt.tile([128, 4, 128], BF16, tag="pst")
                for i, tk in enumerate(range(4 * nt, 4 * nt + 4)):
                    nc.tensor.transpose(
                        pst[:, i, :], vT[b][:, tk * 128:(tk + 1) * 128], ident[:])
                nc.scalar.copy(
                    vA[b][:, 4 * nt:4 * nt + 4, :]
                    .rearrange("p t (h v) -> p t h v", h=2)[:, :, :, 0:64],
                    pst[:].rearrange("p t (h v) -> p t h v", h=2))

            # pipelined emission across the 8 (b, nt) blocks:
            # A(i+1) before B(i)/C(i); D(i-1) after C(i); the x block DMA for
            # i+3 is emitted right after block i's last reader (stage_c)
            pre = {}
            pre[blocks[0]] = stage_a(*blocks[0])
            for i, (b, nt) in enumerate(blocks):
                if i + 1 < len(blocks):
                    pre[blocks[i + 1]] = stage_a(*blocks[i + 1])
                x2t, c1 = pre.pop((b, nt))
                rsig_bc, sd_bf = stage_b(b, nt, x2t, c1)
                stage_c(b, nt, rsig_bc, sd_bf)
                if i + 3 < len(blocks):
                    xts[blocks[i + 3]] = xb_tile(*blocks[i + 3])
                if i > 0:
                    stage_d(*blocks[i - 1])
            stage_d(*blocks[-1])

        # ---- attention + interleaved out-projection ----
        with tc.tile_pool(name="pexp", bufs=4) as exp_pool, \
             tc.tile_pool(name="lnrm", bufs=2) as lnrm, \
             tc.tile_pool(name="psb", bufs=3) as psb, \
             tc.tile_pool(name="ysb", bufs=2) as ysb, \
             tc.tile_pool(name="scp", bufs=2, space="PSUM") as scp, \
             tc.tile_pool(name="psop", bufs=1, space="PSUM") as psop:

            def emit_pv(b, h, pso_bh, pe, g, t, nj):
                for si, (j, off) in enumerate(g):
                    pv_off = max(0, 128 * j - 512 * t)
                    nc.tensor.matmul(
                        pso_bh[:, pv_off:], vA[b][:, j, h * 65:h * 65 + 65],
                        pe[:, si, pv_off:],
                        start=(j == 0), stop=(j == nj - 1), **MM)

            def emit_evict(t, b, pso):
                """ScalarE copies the P@V accumulators to SBUF bf16 - frees
                the PSUM banks for the next unit's P@V immediately."""
                sbs = {}
                for h in range(HL):
                    sb = psb.tile([65, 512], BF16, tag="psosb",
                                  name=f"psosb{h}")
                    nc.scalar.copy(sb[:], pso[h][:])
                    sbs[h] = sb
                return sbs

            def emit_onorm(t, b, h, sb):
                """1/l normalization from the SBUF copy (DVE + GpSimd)."""
                isl = slice(t * 512, (t + 1) * 512)
                hsl = slice(h * 64, (h + 1) * 64)
                lrow = lnrm.tile([1, 512], F32, tag="lrow", bufs=1)
                nc.vector.tensor_copy(lrow[:], sb[64:65, :])
                rec = lnrm.tile([1, 512], F32, tag="rec")
                nc.vector.reciprocal_approx_fast(rec[:], lrow[:])
                lb = lnrm.tile([64, 512], F32, tag="lb")
                nc.gpsimd.partition_broadcast(lb[:], rec[:], channels=64)
                nc.vector.tensor_tensor(
                    oT[b][hsl, isl], sb[0:64, :], lb[:], op=AL.mult)

            def emit_proj(b, tt):
                psy = psop.tile([128, 2, 512], F32, tag="psy", name="psy")
                for half in range(2):
                    for ch in range(2):
                        csl = slice(tt * 128 + 64 * ch,
                                    tt * 128 + 64 * (ch + 1))
                        nc.tensor.matmul(
                            psy[64 * ch:64 * (ch + 1), half, :],
                            oT[b][:, csl],
                            wob[:, half * 512:(half + 1) * 512],
                            start=True, stop=True, **MM)
                yt = ysb.tile([128, D], BF16, tag="yt")
                if tt % 2 == 0:
                    nc.scalar.copy(yt[:], psy[:].rearrange("p a b -> p (a b)"))
                else:
                    nc.vector.tensor_copy(yt[:],
                                          psy[:].rearrange("p a b -> p (a b)"))
                nc.sync.dma_start(y_out.ap()[b, tt * 128:(tt + 1) * 128, :],
                                  yt[:])

            pend_pv = []
            tailq = []        # deferred closures from the previous (t,b) unit
            pend_unit = None  # (t, b, pso) awaiting evict+norm+proj

            def queue_unit_tail(t, b, pso):
                def evict_and_norm():
                    # the unit's trailing P@V matmuls ride late; they must
                    # land before its accumulators are evicted
                    while pend_pv:
                        emit_pv(*pend_pv.pop(0))
                    sbs = emit_evict(t, b, pso)
                    for h in range(HL):
                        emit_onorm(t, b, h, sbs[h])
                tailq.append(evict_and_norm)
                for tt in range(4 * t, 4 * t + 4):
                    tailq.append(lambda b=b, tt=tt: emit_proj(b, tt))

            for t in range(IT):
                nj = 4 * (t + 1)
                bc = bias_tiles.pop(t)
                # pairs over j=0..4t+1 (diagonal tile 4t+1 full width; host
                # bias -1e9 masks it), then two trimmed singles
                groups = [[(2 * k, 0), (2 * k + 1, 0)] for k in range(2 * t + 1)]
                groups.append([(4 * t + 2, 256)])
                groups.append([(4 * t + 3, 384)])
                for b in range(B):
                    if pend_unit is not None:
                        queue_unit_tail(*pend_unit)
                    if tailq:
                        tailq.pop(0)()   # evict+norm of previous unit
                        if tailq:
                            tailq.pop(0)()   # plus one projection: PE food
                                             # while exp(g0) is in flight
                    pso = {h: psop.tile([65, 512], F32, tag=f"pso{h}",
                                        name=f"pso{h}")
                           for h in range(HL)}
                    for gi, g in enumerate(groups):
                        scs = {h: scp.tile([128, 2, 512], F32, tag="scps",
                                           name=f"sc{h}") for h in range(HL)}
                        # scores: h0/h1 adjacent -> concurrent row groups
                        for si, (j, off) in enumerate(g):
                            for h in range(HL):
                                hsl = slice(h * 64, (h + 1) * 64)
                                nc.tensor.matmul(
                                    scs[h][:, si, off:],
                                    kT[b][hsl, j * 128:(j + 1) * 128],
                                    qT[b][hsl, t * 512 + off:(t + 1) * 512],
                                    start=True, stop=False, **MM)
                        # bias adds: 4 cell-disjoint identity tiles per si
                        # (head 0 diagonal halves, head 1 rotated) -> 1 slot
                        for si, (j, off) in enumerate(g):
                            nc.tensor.matmul(
                                scs[0][0:64, si, off:], ident[0:64, 0:64],
                                bc[0:64, 0, j, off:],
                                start=False, stop=False, **MM)
                            nc.tensor.matmul(
                                scs[1][64:128, si, off:], ident[0:64, 0:64],
                                bc[0:64, 1, j, off:],
                                start=False, stop=False, **MM)
                            nc.tensor.matmul(
                                scs[0][64:128, si, off:], ident[64:128, 64:128],
                                bc[64:128, 0, j, off:],
                                start=False, stop=True, **MM)
                            nc.tensor.matmul(
                                scs[1][0:64, si, off:], ident[64:128, 64:128],
                                bc[64:128, 1, j, off:],
                                start=False, stop=True, **MM)
                        for h in range(HL):
                            pe = exp_pool.tile([128, 2, 512], BF16, tag="pe")
                            if len(g) == 2:
                                nc.scalar.activation(pe[:], scs[h][:], AF.Exp)
                            else:
                                off = g[0][1]
                                nc.scalar.activation(pe[:, 0, off:],
                                                     scs[h][:, 0, off:], AF.Exp)
                            pend_pv.append((b, h, pso[h], pe, g, t, nj))
                            if len(pend_pv) > 2:
                                emit_pv(*pend_pv.pop(0))
                        # trickle the previous unit's tail work
                        if gi >= 1 and tailq:
                            tailq.pop(0)()
                            if gi == 1 and tailq:
                                tailq.pop(0)()
                    while tailq:
                        tailq.pop(0)()
                    pend_unit = (t, b, pso)
                # prefetch the bias column two i-tiles ahead; emitted after
                # tile t's bias matmuls so the buffer-reuse write follows all
                # readers of the outgoing tile in program order
                if t + 2 < IT:
                    load_bias(t + 2)
            queue_unit_tail(*pend_unit)
            while tailq:
                tailq.pop(0)()

    nc.compile()
    return nc


_NC_CACHE = {}


def _get_program(bw_zero=True):
    if bw_zero not in _NC_CACHE:
        _NC_CACHE[bw_zero] = build_program(bw_zero)
    return _NC_CACHE[bw_zero]


def build_in_maps(x, attn_bias, ln_gamma, ln_beta, w_qkv, w_out):
    x = np.asarray(x, dtype=np.float32)
    attn_bias = np.asarray(attn_bias, dtype=np.float32)
    ln_gamma = np.asarray(ln_gamma, dtype=np.float32)
    ln_beta = np.asarray(ln_beta, dtype=np.float32)
    w_qkv = np.asarray(w_qkv, dtype=np.float32)
    w_out = np.asarray(w_out, dtype=np.float32)

    ident = np.eye(128, dtype=ml_dtypes.bfloat16)
    xT = np.ascontiguousarray(x.transpose(0, 2, 1)).astype(ml_dtypes.bfloat16)
    # causal mask folded into the bias, transposed to [head, key j, query i]
    tri = np.triu(np.ones((N, N), dtype=bool), k=1)  # True above diag (masked)
    in_maps = []
    for c in range(N_CORES):
        h0 = HL * c
        cols = np.concatenate([
            w_qkv[:, q * H * DH + h0 * DH: q * H * DH + (h0 + HL) * DH]
            for q in range(3)], axis=1)
        # gamma scaling + attention scale on the q block
        cols = cols * ln_gamma[:, None]
        cols[:, 0:128] *= SCALE
        # beta @ W row (before mean-fold; the fold cancels against mu anyway)
        bw = (ln_beta @ cols)[None, :]
        # fold the LN mean subtraction into the weights:
        # x^T (W - 1 colsum(W)/D) = (x - mu)^T W
        cols = cols - cols.sum(axis=0, keepdims=True) / D
        biasT = np.empty((HL, N, N), dtype=ml_dtypes.bfloat16)
        for h in range(HL):
            bh = attn_bias[h0 + h].copy()
            bh[tri] = NEG
            biasT[h] = bh.T.astype(ml_dtypes.bfloat16)
        in_maps.append({
            "xT": xT,
            "biasT": biasT,
            "wqkv": np.ascontiguousarray(cols).astype(ml_dtypes.bfloat16),
            "wout": np.ascontiguousarray(
                w_out[h0 * DH:(h0 + HL) * DH]).astype(ml_dtypes.bfloat16),
            "bw": bw.astype(ml_dtypes.bfloat16),
            "ident": ident,
        })
    return in_maps


def kernel(x, attn_bias, ln_gamma, ln_beta, w_qkv, w_out):
    in_maps = build_in_maps(x, attn_bias, ln_gamma, ln_beta, w_qkv, w_out)
    bw_zero = all(np.all(m["bw"] == 0) for m in in_maps)
    nc = _get_program(bw_zero)
    res = run_bass_kernel_spmd(nc, in_maps, core_ids=list(range(N_CORES)))
    out = np.zeros((B, N, D), dtype=np.float32)
    for c in range(N_CORES):
        out += res.results[c]["y"].astype(np.float32)
    return out


# revision 41
# speedup vs baseline: 1.1338x; 1.1338x over previous
"""Fused LayerNorm + causal multi-head attention (with additive bias) + out-proj
for Trainium2, SPMD over 8 NeuronCores.

Sharding: tensor-parallel over heads. 16 heads / 8 cores = 2 heads per core.
Each core computes LN(x) (replicated), the qkv projection restricted to its
2 heads' columns, causal softmax attention with its heads' bias slices, and a
partial output projection (its heads' rows of w_out). Host sums the 8 partial
outputs (the TP all-reduce, done on gather) in f32 from bf16 partials.

Key choices (engine queues are strict FIFO; emission order is the schedule):
 - x arrives pre-transposed ([dim, token], bf16). The LN mean-subtraction is
   folded into the weights on the HOST: W'' = gW - (1/D) 1 colsum(gW), so
   x^T W'' = (x - mu)^T gW directly. Only rsig (1/std) is applied on device,
   riding the PSUM->SBUF eviction (DVE multiply against a broadcast f32 rsig
   row). The beta term is a rank-1 PSUM matmul (skipped when beta == 0).
 - Input DMAs are emitted in consumption order; x streams through a 3-buffer
   rotating pool of [128, KS, 512] blocks whose refill DMAs are emitted right
   after each block's last reader.
 - PE tile-level concurrency: matmuls whose array footprints are disjoint
   (different 32-row/col groups) execute concurrently. The two heads' C=64
   score matmuls are emitted adjacently -> row groups (0,0)/(64,0) overlap.
   The qkv and out-proj matmuls are emitted as column-split M=64 half pairs
   (tiles (0,0)/(0,64) share the moving operand via two XBUSes) -> 2x.
 - x^2 for the LN variance is computed on the DVE (bf16 tensor-tensor mult
   at 2x rate) to keep the ScalarE queue clear for the short mu^2/sqrt row
   ops on the rsig critical chain.
 - LN stats live in ONE PSUM bank: mean row at partition 0, sum(x^2) row at
   partition 32; the two ones-vector accumulation chains interleave.
 - The causal mask and softmax max-subtraction are folded into the host bias
   (pre-masked with -1e9; logits are O(10) so exp never overflows).
 - The attention bias for each i-tile column is cached in SBUF via one large
   DMA per (head, i-tile) - 8 transfers replace 80 small ones.
 - Batch is OUTER within each i-tile so only one batch's score tiles rotate
   through the 2-buffer score PSUM pool and one batch's P@V accumulators are
   live. Score PSUM tiles span two banks ([128, 2, 512]) so one exp ACTIVATE
   covers two j-tiles; P@V matmuls are emitted two units late so the PE
   never stalls on an exp.
 - Softmax normalization is deferred: an all-ones column appended to V gives
   the row sums l_i for free; the P@V accumulators are evicted to SBUF by
   the ScalarE (fast PSUM free for the next unit), and 1/l is applied from
   SBUF on the DVE/GpSimd. Each unit's normalization + projection trickles
   between the NEXT unit's groups; the projection owns dedicated PSUM banks
   and its evictions alternate ScalarE/VectorE.
"""

import numpy as np
import ml_dtypes
from contextlib import ExitStack

import concourse.bass as bass
import concourse.tile as tile
from concourse import bacc, mybir
from concourse.bass_utils import run_bass_kernel_spmd

F32 = mybir.dt.float32
BF16 = mybir.dt.bfloat16
AL = mybir.AluOpType
AF = mybir.ActivationFunctionType

N_CORES = 8
B = 2            # batch
N = 2048         # tokens
D = 1024         # model dim
H = 16           # total heads
HL = 2           # heads per core
DH = 64          # head dim
COLS = 3 * HL * DH   # 384 qkv columns per core
KS = D // 128    # 8 contraction slabs
TT = N // 128    # 16 token tiles
IT = N // 512    # 4 i-tiles (query tiles of 512)
SCALE = DH ** -0.5
LN_EPS = 1e-5
NEG = -1.0e9


def build_program(bw_zero=True, debug=False):
    nc = bacc.Bacc("TRN2", target_bir_lowering=False, debug=False)

    xT_in = nc.dram_tensor("xT", [B, D, N], BF16, kind="ExternalInput")
    biasT_in = nc.dram_tensor("biasT", [HL, N, N], BF16, kind="ExternalInput")
    wqkv_in = nc.dram_tensor("wqkv", [D, COLS], BF16, kind="ExternalInput")
    wout_in = nc.dram_tensor("wout", [HL * DH, D], BF16, kind="ExternalInput")
    bw_in = nc.dram_tensor("bw", [1, COLS], BF16, kind="ExternalInput")
    ident_in = nc.dram_tensor("ident", [128, 128], BF16, kind="ExternalInput")
    y_out = nc.dram_tensor("y", [B, N, D], BF16, kind="ExternalOutput")

    MM = dict(skip_group_check=True)

    with tile.TileContext(nc) as tc, ExitStack() as ctx:
        # ---- persistent sbuf ----
        pers = ctx.enter_context(tc.tile_pool(name="pers", bufs=1))
        qT = [pers.tile([128, N], BF16, tag=f"qT{b}", name=f"qT{b}") for b in range(B)]
        kT = [pers.tile([128, N], BF16, tag=f"kT{b}", name=f"kT{b}") for b in range(B)]
        vT = [pers.tile([128, N], BF16, tag=f"vT{b}", name=f"vT{b}") for b in range(B)]
        # V natural with ones column: per key-tile [.., 130]: h0 v(64)+1, h1 v(64)+1
        vA = [pers.tile([128, TT, 130], BF16, tag=f"vA{b}", name=f"vA{b}") for b in range(B)]
        oT = [pers.tile([128, N], BF16, tag=f"oT{b}", name=f"oT{b}") for b in range(B)]
        ident = pers.tile([128, 128], BF16, tag="ident")
        wqb = pers.tile([128, KS, COLS], BF16, tag="wqb")
        wob = pers.tile([128, D], BF16, tag="wob")
        onesd = pers.tile([128, 1], BF16, tag="onesd")    # 1/D for stats matmuls
        epsc = pers.tile([128, 1], F32, tag="epsc")

        # attention bias SBUF cache: one i-tile column of all j-tiles, both
        # heads, double-buffered -> 8 large DMAs replace 80 small ones
        bias_pool = ctx.enter_context(tc.tile_pool(name="bias", bufs=2))
        bias_tiles = {}

        def load_bias(t):
            bc = bias_pool.tile([128, HL, TT, 512], BF16, tag="bc",
                                name=f"bc{t}")
            nj = 4 * (t + 1)
            for h in range(HL):
                nc.sync.dma_start(
                    bc[:, h, 0:nj, :],
                    biasT_in.ap()[h, 0:128 * nj, t * 512:(t + 1) * 512]
                    .rearrange("(j p) i -> p j i", p=128))
            bias_tiles[t] = bc

        # ---- input DMAs in consumption order; x streams through a 3-buffer
        # rotating pool (one [128, KS, 512] block per (b, nt)) ----
        xpool = ctx.enter_context(tc.tile_pool(name="xT", bufs=3))

        def xb_tile(b, nt):
            xb = xpool.tile([128, KS, 512], BF16, tag="xb", name=f"xb{b}{nt}")
            sl = slice(nt * 512, (nt + 1) * 512)
            nc.sync.dma_start(
                xb[:], xT_in.ap()[b, :, sl].rearrange("(k p) n -> p k n", p=128))
            return xb

        blocks = [(b, nt) for b in range(B) for nt in range(IT)]
        xts = {}
        xts[blocks[0]] = xb_tile(*blocks[0])
        nc.sync.dma_start(
            wqb[:], wqkv_in.ap().rearrange("(k p) c -> p k c", p=128))
        for blk in blocks[1:3]:
            xts[blk] = xb_tile(*blk)
        load_bias(0)
        nc.sync.dma_start(ident[:], ident_in.ap())
        nc.sync.dma_start(wob[:], wout_in.ap())
        load_bias(1)
        nc.vector.memset(onesd[:], 1.0 / D)
        nc.vector.memset(epsc[:], LN_EPS)
        for b in range(B):
            nc.vector.memset(
                vA[b][:, :, 64::65].rearrange("p t o -> p (t o)"), 1.0)
        if not bw_zero:
            bwb = pers.tile([1, COLS], BF16, tag="bwb")
            nc.sync.dma_start(bwb[:], bw_in.ap())

        # ---- LN stats + qkv^T, per batch; stage-pipelined emission ----
        tree = ctx.enter_context(tc.tile_pool(name="tree", bufs=2))
        rows = ctx.enter_context(tc.tile_pool(name="rows", bufs=2))
        rbc = ctx.enter_context(tc.tile_pool(name="rbc", bufs=2))
        x2p = ctx.enter_context(tc.tile_pool(name="x2p", bufs=2))
        with tc.tile_pool(name="pstat", bufs=1, space="PSUM") as pstat, \
             tc.tile_pool(name="pqkv", bufs=3, space="PSUM") as pqkv, \
             tc.tile_pool(name="pvt", bufs=2, space="PSUM") as pvt:
            dsts = (qT, kT, vT)

            def stage_a(b, nt):
                """x^2 squares (DVE mult) + x slab pre-sum tree (DVE only)."""
                xb = xts[(b, nt)]
                x2t = x2p.tile([128, KS, 512], BF16, tag="x2", name=f"x2_{b}_{nt}")
                nc.vector.tensor_tensor(x2t[:], xb[:], xb[:], op=AL.mult)
                a1 = tree.tile([128, 4, 512], BF16, tag="a1", name=f"a1_{b}_{nt}")
                c1 = tree.tile([128, 2, 512], BF16, tag="c1", name=f"c1_{b}_{nt}")
                nc.vector.tensor_tensor(a1[:], xb[:, 0::2, :], xb[:, 1::2, :],
                                        op=AL.add)
                nc.vector.tensor_tensor(c1[:], a1[:, 0::2, :], a1[:, 1::2, :],
                                        op=AL.add)
                return (x2t, c1)

            def stage_b(b, nt, x2t, c1):
                """stat matmuls: mean row at partition 0, sum(x^2) row at
                partition 32 of ONE psum bank; the two ones-vector chains
                interleave (disjoint col strips)."""
                stat = pstat.tile([128, 512], F32, tag="stat", name="stat")
                mu_ps = stat[0:1, :]
                x2_ps = stat[32:33, :]
                # the has_written clear (start=True) applies to the zero
                # region of the *written* partitions only, so the two chains
                # each open their own group in the shared bank
                nc.tensor.matmul(mu_ps, onesd[:], c1[:, 0, :],
                                 start=True, stop=False, **MM)
                nc.tensor.matmul(x2_ps, onesd[:], x2t[:, 0, :],
                                 start=True, stop=False, **MM)
                nc.tensor.matmul(mu_ps, onesd[:], c1[:, 1, :],
                                 start=False, stop=True, **MM)
                for k in range(1, KS):
                    nc.tensor.matmul(x2_ps, onesd[:], x2t[:, k, :],
                                     start=False, stop=(k == KS - 1), **MM)
                mu2_r = rows.tile([1, 512], F32, tag="mu2_r", bufs=1)
                nc.scalar.activation(mu2_r[:], mu_ps, AF.Square)
                var_r = rows.tile([1, 512], F32, tag="var_r", bufs=1)
                nc.vector.tensor_tensor(var_r[:], x2_ps, mu2_r[:],
                                        op=AL.subtract)
                sd_r = rows.tile([1, 512], F32, tag="sd_r")
                nc.scalar.activation(sd_r[:], var_r[:], AF.Sqrt, bias=epsc[0:1, :])
                rsig_r = rows.tile([1, 512], F32, tag="rsig_r")
                nc.vector.reciprocal_approx_fast(rsig_r[:], sd_r[:])
                rsig_bc = rbc.tile([128, 512], F32, tag="rsbc")
                nc.gpsimd.partition_broadcast(rsig_bc[:], rsig_r[:], channels=128)
                sd_bf = None
                if not bw_zero:
                    sd_bf = rows.tile([1, 512], BF16, tag="sd_bf")
                    nc.vector.tensor_copy(sd_bf[:], sd_r[:])
                return rsig_bc, sd_bf

            def stage_c(b, nt, rsig_bc, sd_bf):
                """qkv matmuls (column-split M=64 concurrent half pairs) +
                rsig eviction to qT/kT/vT."""
                sl = slice(nt * 512, (nt + 1) * 512)
                xb = xts[(b, nt)]
                for blk in range(3):
                    ps = pqkv.tile([128, 512], F32, tag="psqkv")
                    for k in range(KS):
                        for hf in range(2):
                            cs = slice(blk * 128 + 64 * hf,
                                       blk * 128 + 64 * (hf + 1))
                            nc.tensor.matmul(
                                ps[64 * hf:64 * (hf + 1), :],
                                wqb[:, k, cs], xb[:, k, :],
                                start=(k == 0),
                                stop=(k == KS - 1 and bw_zero), **MM)
                    if not bw_zero:
                        csl = slice(blk * 128, (blk + 1) * 128)
                        nc.tensor.matmul(ps[:], bwb[0:1, csl], sd_bf[0:1, :],
                                         start=False, stop=True, **MM)
                    nc.vector.tensor_tensor(dsts[blk][b][:, sl], ps[:],
                                            rsig_bc[:], op=AL.mult)

            def stage_d(b, nt):
                """V -> natural layout: 4 PE transposes + one ScalarE copy."""
                pst = pvt.tile([128, 4, 128], BF16, tag="pst")
                for i, tk in enumerate(range(4 * nt, 4 * nt + 4)):
                    nc.tensor.transpose(
                        pst[:, i, :], vT[b][:, tk * 128:(tk + 1) * 128], ident[:])
                nc.scalar.copy(
                    vA[b][:, 4 * nt:4 * nt + 4, :]
                    .rearrange("p t (h v) -> p t h v", h=2)[:, :, :, 0:64],
                    pst[:].rearrange("p t (h v) -> p t h v", h=2))

            # pipelined emission across the 8 (b, nt) blocks:
            # A(i+1) before B(i)/C(i); D(i-1) after C(i); the x block DMA for
            # i+3 is emitted right after block i's last reader (stage_c)
            pre = {}
            pre[blocks[0]] = stage_a(*blocks[0])
            for i, (b, nt) in enumerate(blocks):
                if i + 1 < len(blocks):
                    pre[blocks[i + 1]] = stage_a(*blocks[i + 1])
                x2t, c1 = pre.pop((b, nt))
                rsig_bc, sd_bf = stage_b(b, nt, x2t, c1)
                stage_c(b, nt, rsig_bc, sd_bf)
                if i + 3 < len(blocks):
                    xts[blocks[i + 3]] = xb_tile(*blocks[i + 3])
                if i > 0:
                    stage_d(*blocks[i - 1])
            stage_d(*blocks[-1])

        # ---- attention + interleaved out-projection ----
        with tc.tile_pool(name="pexp", bufs=4) as exp_pool, \
             tc.tile_pool(name="lnrm", bufs=2) as lnrm, \
             tc.tile_pool(name="psb", bufs=3) as psb, \
             tc.tile_pool(name="ysb", bufs=2) as ysb, \
             tc.tile_pool(name="scp", bufs=2, space="PSUM") as scp, \
             tc.tile_pool(name="psop", bufs=1, space="PSUM") as psop:

            def emit_pv(b, h, pso_bh, pe, g, t, nj):
                for si, (j, off) in enumerate(g):
                    pv_off = max(0, 128 * j - 512 * t)
                    nc.tensor.matmul(
                        pso_bh[:, pv_off:], vA[b][:, j, h * 65:h * 65 + 65],
                        pe[:, si, pv_off:],
                        start=(j == 0), stop=(j == nj - 1), **MM)

            def emit_evict(t, b, pso):
                """ScalarE copies the P@V accumulators to SBUF bf16 - frees
                the PSUM banks for the next unit's P@V immediately."""
                sbs = {}
                for h in range(HL):
                    sb = psb.tile([65, 512], BF16, tag="psosb",
                                  name=f"psosb{h}")
                    nc.scalar.copy(sb[:], pso[h][:])
                    sbs[h] = sb
                return sbs

            def emit_onorm(t, b, h, sb):
                """1/l normalization from the SBUF copy (DVE + GpSimd)."""
                isl = slice(t * 512, (t + 1) * 512)
                hsl = slice(h * 64, (h + 1) * 64)
                lrow = lnrm.tile([1, 512], F32, tag="lrow", bufs=1)
                nc.vector.tensor_copy(lrow[:], sb[64:65, :])
                rec = lnrm.tile([1, 512], F32, tag="rec")
                nc.vector.reciprocal_approx_fast(rec[:], lrow[:])
                lb = lnrm.tile([64, 512], F32, tag="lb")
                nc.gpsimd.partition_broadcast(lb[:], rec[:], channels=64)
                nc.vector.tensor_tensor(
                    oT[b][hsl, isl], sb[0:64, :], lb[:], op=AL.mult)

            def emit_proj(b, tt):
                psy = psop.tile([128, 2, 512], F32, tag="psy", name="psy")
                for half in range(2):
                    for ch in range(2):
                        csl = slice(tt * 128 + 64 * ch,
                                    tt * 128 + 64 * (ch + 1))
                        nc.tensor.matmul(
                            psy[64 * ch:64 * (ch + 1), half, :],
                            oT[b][:, csl],
                            wob[:, half * 512:(half + 1) * 512],
                            start=True, stop=True, **MM)
                yt = ysb.tile([128, D], BF16, tag="yt")
                if tt % 2 == 0:
                    nc.scalar.copy(yt[:], psy[:].rearrange("p a b -> p (a b)"))
                else:
                    nc.vector.tensor_copy(yt[:],
                                          psy[:].rearrange("p a b -> p (a b)"))
                nc.sync.dma_start(y_out.ap()[b, tt * 128:(tt + 1) * 128, :],
                                  yt[:])

            pend_pv = []
            tailq = []        # deferred closures from the previous (t,b) unit
            pend_unit = None  # (t, b, pso) awaiting evict+norm+proj

            def queue_unit_tail(t, b, pso):
                def evict_and_norm():
                    # the unit's trailing P@V matmuls ride late; they must
                    # land before its accumulators are evicted
                    while pend_pv:
                        emit_pv(*pend_pv.pop(0))
                    sbs = emit_evict(t, b, pso)
                    for h in range(HL):
                        emit_onorm(t, b, h, sbs[h])
                tailq.append(evict_and_norm)
                for tt in range(4 * t, 4 * t + 4):
                    tailq.append(lambda b=b, tt=tt: emit_proj(b, tt))

            for t in range(IT):
                nj = 4 * (t + 1)
                bc = bias_tiles.pop(t)
                # pairs over j=0..4t+1 (diagonal tile 4t+1 full width; host
                # bias -1e9 masks it), then two trimmed singles
                groups = [[(2 * k, 0), (2 * k + 1, 0)] for k in range(2 * t + 1)]
                # the two trimmed outer-diagonal tiles merge into one
                # 256-offset group (j=4t+3's extra quarter is host-masked)
                groups.append([(4 * t + 2, 256), (4 * t + 3, 256)])
                for b in range(B):
                    if pend_unit is not None:
                        queue_unit_tail(*pend_unit)
                    if tailq:
                        tailq.pop(0)()   # evict+norm of previous unit
                        if tailq:
                            tailq.pop(0)()   # plus one projection: PE food
                                             # while exp(g0) is in flight
                    pso = {h: psop.tile([65, 512], F32, tag=f"pso{h}",
                                        name=f"pso{h}")
                           for h in range(HL)}
                    for gi, g in enumerate(groups):
                        scs = {h: scp.tile([128, 2, 512], F32, tag="scps",
                                           name=f"sc{h}") for h in range(HL)}
                        # scores: h0/h1 adjacent -> concurrent row groups
                        for si, (j, off) in enumerate(g):
                            for h in range(HL):
                                hsl = slice(h * 64, (h + 1) * 64)
                                nc.tensor.matmul(
                                    scs[h][:, si, off:],
                                    kT[b][hsl, j * 128:(j + 1) * 128],
                                    qT[b][hsl, t * 512 + off:(t + 1) * 512],
                                    start=True, stop=False, **MM)
                        # bias adds (identity matmuls) from the SBUF cache
                        for si, (j, off) in enumerate(g):
                            for h in range(HL):
                                nc.tensor.matmul(
                                    scs[h][:, si, off:], ident[:],
                                    bc[:, h, j, off:],
                                    start=False, stop=True, **MM)
                        offm = g[0][1]
                        for h in range(HL):
                            pe = exp_pool.tile([128, 2, 512], BF16, tag="pe")
                            nc.scalar.activation(pe[:, :, offm:],
                                                 scs[h][:, :, offm:], AF.Exp)
                            pend_pv.append((b, h, pso[h], pe, g, t, nj))
                            if len(pend_pv) > 2:
                                emit_pv(*pend_pv.pop(0))
                        # trickle the previous unit's tail work
                        if gi >= 1 and tailq:
                            tailq.pop(0)()
                            if gi == 1 and tailq:
                                tailq.pop(0)()
                    while tailq:
                        tailq.pop(0)()
                    pend_unit = (t, b, pso)
                # prefetch the bias column two i-tiles ahead; emitted after
                # tile t's bias matmuls so the buffer-reuse write follows all
                # readers of the outgoing tile in program order
                if t + 2 < IT:
                    load_bias(t + 2)
            queue_unit_tail(*pend_unit)
            while tailq:
                tailq.pop(0)()

    nc.compile()
    return nc


_NC_CACHE = {}


def _get_program(bw_zero=True):
    if bw_zero not in _NC_CACHE:
        _NC_CACHE[bw_zero] = build_program(bw_zero)
    return _NC_CACHE[bw_zero]


def build_in_maps(x, attn_bias, ln_gamma, ln_beta, w_qkv, w_out):
    x = np.asarray(x, dtype=np.float32)
    attn_bias = np.asarray(attn_bias, dtype=np.float32)
    ln_gamma = np.asarray(ln_gamma, dtype=np.float32)
    ln_beta = np.asarray(ln_beta, dtype=np.float32)
    w_qkv = np.asarray(w_qkv, dtype=np.float32)
    w_out = np.asarray(w_out, dtype=np.float32)

    ident = np.eye(128, dtype=ml_dtypes.bfloat16)
    xT = np.ascontiguousarray(x.transpose(0, 2, 1)).astype(ml_dtypes.bfloat16)
    # causal mask folded into the bias, transposed to [head, key j, query i]
    tri = np.triu(np.ones((N, N), dtype=bool), k=1)  # True above diag (masked)
    in_maps = []
    for c in range(N_CORES):
        h0 = HL * c
        cols = np.concatenate([
            w_qkv[:, q * H * DH + h0 * DH: q * H * DH + (h0 + HL) * DH]
            for q in range(3)], axis=1)
        # gamma scaling + attention scale on the q block
        cols = cols * ln_gamma[:, None]
        cols[:, 0:128] *= SCALE
        # beta @ W row (before mean-fold; the fold cancels against mu anyway)
        bw = (ln_beta @ cols)[None, :]
        # fold the LN mean subtraction into the weights:
        # x^T (W - 1 colsum(W)/D) = (x - mu)^T W
        cols = cols - cols.sum(axis=0, keepdims=True) / D
        biasT = np.empty((HL, N, N), dtype=ml_dtypes.bfloat16)
        for h in range(HL):
            bh = attn_bias[h0 + h].copy()
            bh[tri] = NEG
            biasT[h] = bh.T.astype(ml_dtypes.bfloat16)
        in_maps.append({
            "xT": xT,
            "biasT": biasT,
            "wqkv": np.ascontiguousarray(cols).astype(ml_dtypes.bfloat16),
            "wout": np.ascontiguousarray(
                w_out[h0 * DH:(h0 + HL) * DH]).astype(ml_dtypes.bfloat16),
            "bw": bw.astype(ml_dtypes.bfloat16),
            "ident": ident,
        })
    return in_maps


def kernel(x, attn_bias, ln_gamma, ln_beta, w_qkv, w_out):
    in_maps = build_in_maps(x, attn_bias, ln_gamma, ln_beta, w_qkv, w_out)
    bw_zero = all(np.all(m["bw"] == 0) for m in in_maps)
    nc = _get_program(bw_zero)
    res = run_bass_kernel_spmd(nc, in_maps, core_ids=list(range(N_CORES)))
    out = np.zeros((B, N, D), dtype=np.float32)
    for c in range(N_CORES):
        out += res.results[c]["y"].astype(np.float32)
    return out


# revision 42
# speedup vs baseline: 1.1512x; 1.0153x over previous
"""Fused LayerNorm + causal multi-head attention (with additive bias) + out-proj
for Trainium2, SPMD over 8 NeuronCores.

Sharding: tensor-parallel over heads. 16 heads / 8 cores = 2 heads per core.
Each core computes LN(x) (replicated), the qkv projection restricted to its
2 heads' columns, causal softmax attention with its heads' bias slices, and a
partial output projection (its heads' rows of w_out). Host sums the 8 partial
outputs (the TP all-reduce, done on gather) in f32 from bf16 partials.

Key choices (engine queues are strict FIFO; emission order is the schedule):
 - x arrives pre-transposed ([dim, token], bf16). The LN mean-subtraction is
   folded into the weights on the HOST: W'' = gW - (1/D) 1 colsum(gW), so
   x^T W'' = (x - mu)^T gW directly. Only rsig (1/std) is applied on device,
   riding the PSUM->SBUF eviction (DVE multiply against a broadcast f32 rsig
   row). The beta term is a rank-1 PSUM matmul (skipped when beta == 0).
 - Input DMAs are emitted in consumption order; x streams through a 3-buffer
   rotating pool of [128, KS, 512] blocks whose refill DMAs are emitted right
   after each block's last reader.
 - PE tile-level concurrency: matmuls whose array footprints are disjoint
   (different 32-row/col groups) execute concurrently. The two heads' C=64
   score matmuls are emitted adjacently -> row groups (0,0)/(64,0) overlap.
   The qkv and out-proj matmuls are emitted as column-split M=64 half pairs
   (tiles (0,0)/(0,64) share the moving operand via two XBUSes) -> 2x.
 - x^2 for the LN variance is computed on the DVE (bf16 tensor-tensor mult
   at 2x rate) to keep the ScalarE queue clear for the short mu^2/sqrt row
   ops on the rsig critical chain.
 - LN stats live in ONE PSUM bank: mean row at partition 0, sum(x^2) row at
   partition 32; the two ones-vector accumulation chains interleave.
 - The causal mask and softmax max-subtraction are folded into the host bias
   (pre-masked with -1e9; logits are O(10) so exp never overflows).
 - The attention bias for each i-tile column is cached in SBUF via one large
   DMA per (head, i-tile) - 8 transfers replace 80 small ones.
 - Batch is OUTER within each i-tile so only one batch's score tiles rotate
   through the 2-buffer score PSUM pool and one batch's P@V accumulators are
   live. Score PSUM tiles span two banks ([128, 2, 512]) so one exp ACTIVATE
   covers two j-tiles; P@V matmuls are emitted two units late so the PE
   never stalls on an exp.
 - Softmax normalization is deferred: an all-ones column appended to V gives
   the row sums l_i for free; the P@V accumulators are evicted to SBUF by
   the ScalarE (fast PSUM free for the next unit), and 1/l is applied from
   SBUF on the DVE/GpSimd. Each unit's normalization + projection trickles
   between the NEXT unit's groups; the projection owns dedicated PSUM banks
   and its evictions alternate ScalarE/VectorE.
"""

import numpy as np
import ml_dtypes
from contextlib import ExitStack

import concourse.bass as bass
import concourse.tile as tile
from concourse import bacc, mybir
from concourse.bass_utils import run_bass_kernel_spmd

F32 = mybir.dt.float32
BF16 = mybir.dt.bfloat16
AL = mybir.AluOpType
AF = mybir.ActivationFunctionType

N_CORES = 8
B = 2            # batch
N = 2048         # tokens
D = 1024         # model dim
H = 16           # total heads
HL = 2           # heads per core
DH = 64          # head dim
COLS = 3 * HL * DH   # 384 qkv columns per core
KS = D // 128    # 8 contraction slabs
TT = N // 128    # 16 token tiles
IT = N // 512    # 4 i-tiles (query tiles of 512)
SCALE = DH ** -0.5
LN_EPS = 1e-5
NEG = -1.0e9


def build_program(bw_zero=True, debug=False):
    nc = bacc.Bacc("TRN2", target_bir_lowering=False, debug=False)

    xT_in = nc.dram_tensor("xT", [B, D, N], BF16, kind="ExternalInput")
    biasT_in = nc.dram_tensor("biasT", [HL, N, N], BF16, kind="ExternalInput")
    wqkv_in = nc.dram_tensor("wqkv", [D, COLS], BF16, kind="ExternalInput")
    wout_in = nc.dram_tensor("wout", [HL * DH, D], BF16, kind="ExternalInput")
    bw_in = nc.dram_tensor("bw", [1, COLS], BF16, kind="ExternalInput")
    ident_in = nc.dram_tensor("ident", [128, 128], BF16, kind="ExternalInput")
    y_out = nc.dram_tensor("y", [B, N, D], BF16, kind="ExternalOutput")

    MM = dict(skip_group_check=True)

    with tile.TileContext(nc) as tc, ExitStack() as ctx:
        # ---- persistent sbuf ----
        pers = ctx.enter_context(tc.tile_pool(name="pers", bufs=1))
        qT = [pers.tile([128, N], BF16, tag=f"qT{b}", name=f"qT{b}") for b in range(B)]
        kT = [pers.tile([128, N], BF16, tag=f"kT{b}", name=f"kT{b}") for b in range(B)]
        vT = [pers.tile([128, N], BF16, tag=f"vT{b}", name=f"vT{b}") for b in range(B)]
        # V natural with ones column: per key-tile [.., 130]: h0 v(64)+1, h1 v(64)+1
        vA = [pers.tile([128, TT, 130], BF16, tag=f"vA{b}", name=f"vA{b}") for b in range(B)]
        oT = [pers.tile([128, N], BF16, tag=f"oT{b}", name=f"oT{b}") for b in range(B)]
        ident = pers.tile([128, 128], BF16, tag="ident")
        wqb = pers.tile([128, KS, COLS], BF16, tag="wqb")
        wob = pers.tile([128, D], BF16, tag="wob")
        onesd = pers.tile([128, 1], BF16, tag="onesd")    # 1/D for stats matmuls
        epsc = pers.tile([128, 1], F32, tag="epsc")

        # attention bias SBUF cache: one i-tile column of all j-tiles, both
        # heads, double-buffered -> 8 large DMAs replace 80 small ones
        bias_pool = ctx.enter_context(tc.tile_pool(name="bias", bufs=2))
        bias_tiles = {}

        def load_bias(t):
            bc = bias_pool.tile([128, HL, TT, 512], BF16, tag="bc",
                                name=f"bc{t}")
            nj = 4 * (t + 1)
            for h in range(HL):
                nc.sync.dma_start(
                    bc[:, h, 0:nj, :],
                    biasT_in.ap()[h, 0:128 * nj, t * 512:(t + 1) * 512]
                    .rearrange("(j p) i -> p j i", p=128))
            bias_tiles[t] = bc

        # ---- input DMAs in consumption order; x streams through a 3-buffer
        # rotating pool (one [128, KS, 512] block per (b, nt)) ----
        xpool = ctx.enter_context(tc.tile_pool(name="xT", bufs=3))

        def xb_tile(b, nt):
            xb = xpool.tile([128, KS, 512], BF16, tag="xb", name=f"xb{b}{nt}")
            sl = slice(nt * 512, (nt + 1) * 512)
            nc.sync.dma_start(
                xb[:], xT_in.ap()[b, :, sl].rearrange("(k p) n -> p k n", p=128))
            return xb

        blocks = [(b, nt) for b in range(B) for nt in range(IT)]
        xts = {}
        xts[blocks[0]] = xb_tile(*blocks[0])
        nc.sync.dma_start(
            wqb[:], wqkv_in.ap().rearrange("(k p) c -> p k c", p=128))
        for blk in blocks[1:3]:
            xts[blk] = xb_tile(*blk)
        load_bias(0)
        nc.sync.dma_start(ident[:], ident_in.ap())
        nc.sync.dma_start(wob[:], wout_in.ap())
        load_bias(1)
        nc.vector.memset(onesd[:], 1.0 / D)
        nc.vector.memset(epsc[:], LN_EPS)
        for b in range(B):
            nc.vector.memset(
                vA[b][:, :, 64::65].rearrange("p t o -> p (t o)"), 1.0)
        if not bw_zero:
            bwb = pers.tile([1, COLS], BF16, tag="bwb")
            nc.sync.dma_start(bwb[:], bw_in.ap())

        # ---- LN stats + qkv^T, per batch; stage-pipelined emission ----
        tree = ctx.enter_context(tc.tile_pool(name="tree", bufs=2))
        rows = ctx.enter_context(tc.tile_pool(name="rows", bufs=2))
        rbc = ctx.enter_context(tc.tile_pool(name="rbc", bufs=2))
        x2p = ctx.enter_context(tc.tile_pool(name="x2p", bufs=2))
        with tc.tile_pool(name="pstat", bufs=1, space="PSUM") as pstat, \
             tc.tile_pool(name="pqkv", bufs=3, space="PSUM") as pqkv, \
             tc.tile_pool(name="pvt", bufs=2, space="PSUM") as pvt:
            dsts = (qT, kT, vT)

            def stage_a(b, nt):
                """x^2 squares (DVE mult) + x slab pre-sum tree (DVE only)."""
                xb = xts[(b, nt)]
                x2t = x2p.tile([128, KS, 512], BF16, tag="x2", name=f"x2_{b}_{nt}")
                nc.vector.tensor_tensor(x2t[:], xb[:], xb[:], op=AL.mult)
                a1 = tree.tile([128, 4, 512], BF16, tag="a1", name=f"a1_{b}_{nt}")
                c1 = tree.tile([128, 2, 512], BF16, tag="c1", name=f"c1_{b}_{nt}")
                nc.vector.tensor_tensor(a1[:], xb[:, 0::2, :], xb[:, 1::2, :],
                                        op=AL.add)
                nc.vector.tensor_tensor(c1[:], a1[:, 0::2, :], a1[:, 1::2, :],
                                        op=AL.add)
                return (x2t, c1)

            def stage_b(b, nt, x2t, c1):
                """stat matmuls: mean row at partition 0, sum(x^2) row at
                partition 32 of ONE psum bank; the two ones-vector chains
                interleave (disjoint col strips)."""
                stat = pstat.tile([128, 512], F32, tag="stat", name="stat")
                mu_ps = stat[0:1, :]
                x2_ps = stat[32:33, :]
                # the has_written clear (start=True) applies to the zero
                # region of the *written* partitions only, so the two chains
                # each open their own group in the shared bank
                nc.tensor.matmul(mu_ps, onesd[:], c1[:, 0, :],
                                 start=True, stop=False, **MM)
                nc.tensor.matmul(x2_ps, onesd[:], x2t[:, 0, :],
                                 start=True, stop=False, **MM)
                nc.tensor.matmul(mu_ps, onesd[:], c1[:, 1, :],
                                 start=False, stop=True, **MM)
                for k in range(1, KS):
                    nc.tensor.matmul(x2_ps, onesd[:], x2t[:, k, :],
                                     start=False, stop=(k == KS - 1), **MM)
                mu2_r = rows.tile([1, 512], F32, tag="mu2_r", bufs=1)
                nc.scalar.activation(mu2_r[:], mu_ps, AF.Square)
                var_r = rows.tile([1, 512], F32, tag="var_r", bufs=1)
                nc.vector.tensor_tensor(var_r[:], x2_ps, mu2_r[:],
                                        op=AL.subtract)
                sd_r = rows.tile([1, 512], F32, tag="sd_r")
                nc.scalar.activation(sd_r[:], var_r[:], AF.Sqrt, bias=epsc[0:1, :])
                rsig_r = rows.tile([1, 512], F32, tag="rsig_r")
                nc.vector.reciprocal_approx_fast(rsig_r[:], sd_r[:])
                rsig_bc = rbc.tile([128, 512], F32, tag="rsbc")
                nc.gpsimd.partition_broadcast(rsig_bc[:], rsig_r[:], channels=128)
                sd_bf = None
                if not bw_zero:
                    sd_bf = rows.tile([1, 512], BF16, tag="sd_bf")
                    nc.vector.tensor_copy(sd_bf[:], sd_r[:])
                return rsig_bc, sd_bf

            def stage_c(b, nt, rsig_bc, sd_bf):
                """qkv matmuls (column-split M=64 concurrent half pairs) +
                rsig eviction to qT/kT/vT."""
                sl = slice(nt * 512, (nt + 1) * 512)
                xb = xts[(b, nt)]
                for blk in range(3):
                    ps = pqkv.tile([128, 512], F32, tag="psqkv")
                    for k in range(KS):
                        for hf in range(2):
                            cs = slice(blk * 128 + 64 * hf,
                                       blk * 128 + 64 * (hf + 1))
                            nc.tensor.matmul(
                                ps[64 * hf:64 * (hf + 1), :],
                                wqb[:, k, cs], xb[:, k, :],
                                start=(k == 0),
                                stop=(k == KS - 1 and bw_zero), **MM)
                    if not bw_zero:
                        csl = slice(blk * 128, (blk + 1) * 128)
                        nc.tensor.matmul(ps[:], bwb[0:1, csl], sd_bf[0:1, :],
                                         start=False, stop=True, **MM)
                    nc.vector.tensor_tensor(dsts[blk][b][:, sl], ps[:],
                                            rsig_bc[:], op=AL.mult)

            def stage_d(b, nt):
                """V -> natural layout: 4 PE transposes + one ScalarE copy."""
                pst = pvt.tile([128, 4, 128], BF16, tag="pst")
                for i, tk in enumerate(range(4 * nt, 4 * nt + 4)):
                    nc.tensor.transpose(
                        pst[:, i, :], vT[b][:, tk * 128:(tk + 1) * 128], ident[:])
                nc.scalar.copy(
                    vA[b][:, 4 * nt:4 * nt + 4, :]
                    .rearrange("p t (h v) -> p t h v", h=2)[:, :, :, 0:64],
                    pst[:].rearrange("p t (h v) -> p t h v", h=2))

            # pipelined emission across the 8 (b, nt) blocks:
            # A(i+1) before B(i)/C(i); D(i-1) after C(i); the x block DMA for
            # i+3 is emitted right after block i's last reader (stage_c)
            pre = {}
            pre[blocks[0]] = stage_a(*blocks[0])
            for i, (b, nt) in enumerate(blocks):
                if i + 1 < len(blocks):
                    pre[blocks[i + 1]] = stage_a(*blocks[i + 1])
                x2t, c1 = pre.pop((b, nt))
                rsig_bc, sd_bf = stage_b(b, nt, x2t, c1)
                stage_c(b, nt, rsig_bc, sd_bf)
                if i + 3 < len(blocks):
                    xts[blocks[i + 3]] = xb_tile(*blocks[i + 3])
                if i > 0:
                    stage_d(*blocks[i - 1])
            stage_d(*blocks[-1])

        # ---- attention + interleaved out-projection ----
        with tc.tile_pool(name="pexp", bufs=4) as exp_pool, \
             tc.tile_pool(name="lnrm", bufs=2) as lnrm, \
             tc.tile_pool(name="psb", bufs=3) as psb, \
             tc.tile_pool(name="ysb", bufs=2) as ysb, \
             tc.tile_pool(name="scp", bufs=2, space="PSUM") as scp, \
             tc.tile_pool(name="psop", bufs=1, space="PSUM") as psop:

            def emit_pv(b, h, pso_bh, pe, g, t, nj):
                for si, (j, off) in enumerate(g):
                    pv_off = max(0, 128 * j - 512 * t)
                    nc.tensor.matmul(
                        pso_bh[:, pv_off:], vA[b][:, j, h * 65:h * 65 + 65],
                        pe[:, si, pv_off:],
                        start=(j == 0), stop=(j == nj - 1), **MM)

            def emit_evict(t, b, pso):
                """ScalarE copies the P@V accumulators to SBUF bf16 - frees
                the PSUM banks for the next unit's P@V immediately."""
                sbs = {}
                for h in range(HL):
                    sb = psb.tile([65, 512], BF16, tag="psosb",
                                  name=f"psosb{h}")
                    nc.scalar.copy(sb[:], pso[h][:])
                    sbs[h] = sb
                return sbs

            def emit_onorm(t, b, h, sb):
                """1/l normalization from the SBUF copy (DVE + GpSimd)."""
                isl = slice(t * 512, (t + 1) * 512)
                hsl = slice(h * 64, (h + 1) * 64)
                lrow = lnrm.tile([1, 512], F32, tag="lrow", bufs=1)
                nc.vector.tensor_copy(lrow[:], sb[64:65, :])
                rec = lnrm.tile([1, 512], F32, tag="rec")
                nc.vector.reciprocal_approx_fast(rec[:], lrow[:])
                lb = lnrm.tile([64, 512], F32, tag="lb")
                nc.gpsimd.partition_broadcast(lb[:], rec[:], channels=64)
                nc.vector.tensor_tensor(
                    oT[b][hsl, isl], sb[0:64, :], lb[:], op=AL.mult)

            def emit_proj(b, tt):
                psy = psop.tile([128, 2, 512], F32, tag="psy", name="psy")
                for half in range(2):
                    for ch in range(2):
                        csl = slice(tt * 128 + 64 * ch,
                                    tt * 128 + 64 * (ch + 1))
                        nc.tensor.matmul(
                            psy[64 * ch:64 * (ch + 1), half, :],
                            oT[b][:, csl],
                            wob[:, half * 512:(half + 1) * 512],
                            start=True, stop=True, **MM)
                yt = ysb.tile([128, D], BF16, tag="yt")
                if tt % 2 == 0:
                    nc.scalar.copy(yt[:], psy[:].rearrange("p a b -> p (a b)"))
                else:
                    nc.vector.tensor_copy(yt[:],
                                          psy[:].rearrange("p a b -> p (a b)"))
                nc.sync.dma_start(y_out.ap()[b, tt * 128:(tt + 1) * 128, :],
                                  yt[:])

            pend_pv = []
            tailq = []        # deferred closures from the previous (t,b) unit
            pend_unit = None  # (t, b, pso) awaiting evict+norm+proj

            def queue_unit_tail(t, b, pso):
                def evict_and_norm():
                    # the unit's trailing P@V matmuls ride late; they must
                    # land before its accumulators are evicted
                    while pend_pv:
                        emit_pv(*pend_pv.pop(0))
                    sbs = emit_evict(t, b, pso)
                    for h in range(HL):
                        emit_onorm(t, b, h, sbs[h])
                tailq.append(evict_and_norm)
                for tt in range(4 * t, 4 * t + 4):
                    tailq.append(lambda b=b, tt=tt: emit_proj(b, tt))

            for t in range(IT):
                nj = 4 * (t + 1)
                bc = bias_tiles.pop(t)
                # pairs over j=0..4t+1 (diagonal tile 4t+1 full width; host
                # bias -1e9 masks it), then two trimmed singles
                groups = [[(2 * k, 0), (2 * k + 1, 0)] for k in range(2 * t + 1)]
                groups.append([(4 * t + 2, 256)])
                groups.append([(4 * t + 3, 384)])
                for b in range(B):
                    if pend_unit is not None:
                        queue_unit_tail(*pend_unit)
                    if tailq:
                        tailq.pop(0)()   # evict+norm of previous unit
                    pso = {h: psop.tile([65, 512], F32, tag=f"pso{h}",
                                        name=f"pso{h}")
                           for h in range(HL)}
                    for gi, g in enumerate(groups):
                        scs = {h: scp.tile([128, 2, 512], F32, tag="scps",
                                           name=f"sc{h}") for h in range(HL)}
                        # scores: h0/h1 adjacent -> concurrent row groups
                        for si, (j, off) in enumerate(g):
                            for h in range(HL):
                                hsl = slice(h * 64, (h + 1) * 64)
                                nc.tensor.matmul(
                                    scs[h][:, si, off:],
                                    kT[b][hsl, j * 128:(j + 1) * 128],
                                    qT[b][hsl, t * 512 + off:(t + 1) * 512],
                                    start=True, stop=False, **MM)
                        # bias adds (identity matmuls) from the SBUF cache
                        for si, (j, off) in enumerate(g):
                            for h in range(HL):
                                nc.tensor.matmul(
                                    scs[h][:, si, off:], ident[:],
                                    bc[:, h, j, off:],
                                    start=False, stop=True, **MM)
                        for h in range(HL):
                            pe = exp_pool.tile([128, 2, 512], BF16, tag="pe")
                            if len(g) == 2:
                                nc.scalar.activation(pe[:], scs[h][:], AF.Exp)
                            else:
                                off = g[0][1]
                                nc.scalar.activation(pe[:, 0, off:],
                                                     scs[h][:, 0, off:], AF.Exp)
                            pend_pv.append((b, h, pso[h], pe, g, t, nj))
                            if len(pend_pv) > 2:
                                emit_pv(*pend_pv.pop(0))
                        # trickle the previous unit's tail work
                        if gi >= 1 and tailq:
                            tailq.pop(0)()
                            if gi == 1 and tailq:
                                tailq.pop(0)()
                    while tailq:
                        tailq.pop(0)()
                    pend_unit = (t, b, pso)
                # prefetch the bias column two i-tiles ahead; emitted after
                # tile t's bias matmuls so the buffer-reuse write follows all
                # readers of the outgoing tile in program order
                if t + 2 < IT:
                    load_bias(t + 2)
            queue_unit_tail(*pend_unit)
            while tailq:
                tailq.pop(0)()

    nc.compile()
    return nc


_NC_CACHE = {}


def _get_program(bw_zero=True):
    if bw_zero not in _NC_CACHE:
        _NC_CACHE[bw_zero] = build_program(bw_zero)
    return _NC_CACHE[bw_zero]


def build_in_maps(x, attn_bias, ln_gamma, ln_beta, w_qkv, w_out):
    x = np.asarray(x, dtype=np.float32)
    attn_bias = np.asarray(attn_bias, dtype=np.float32)
    ln_gamma = np.asarray(ln_gamma, dtype=np.float32)
    ln_beta = np.asarray(ln_beta, dtype=np.float32)
    w_qkv = np.asarray(w_qkv, dtype=np.float32)
    w_out = np.asarray(w_out, dtype=np.float32)

    ident = np.eye(128, dtype=ml_dtypes.bfloat16)
    xT = np.ascontiguousarray(x.transpose(0, 2, 1)).astype(ml_dtypes.bfloat16)
    # causal mask folded into the bias, transposed to [head, key j, query i]
    tri = np.triu(np.ones((N, N), dtype=bool), k=1)  # True above diag (masked)
    in_maps = []
    for c in range(N_CORES):
        h0 = HL * c
        cols = np.concatenate([
            w_qkv[:, q * H * DH + h0 * DH: q * H * DH + (h0 + HL) * DH]
            for q in range(3)], axis=1)
        # gamma scaling + attention scale on the q block
        cols = cols * ln_gamma[:, None]
        cols[:, 0:128] *= SCALE
        # beta @ W row (before mean-fold; the fold cancels against mu anyway)
        bw = (ln_beta @ cols)[None, :]
        # fold the LN mean subtraction into the weights:
        # x^T (W - 1 colsum(W)/D) = (x - mu)^T W
        cols = cols - cols.sum(axis=0, keepdims=True) / D
        biasT = np.empty((HL, N, N), dtype=ml_dtypes.bfloat16)
        for h in range(HL):
            bh = attn_bias[h0 + h].copy()
            bh[tri] = NEG
            biasT[h] = bh.T.astype(ml_dtypes.bfloat16)
        in_maps.append({
            "xT": xT,
            "biasT": biasT,
            "wqkv": np.ascontiguousarray(cols).astype(ml_dtypes.bfloat16),
            "wout": np.ascontiguousarray(
                w_out[h0 * DH:(h0 + HL) * DH]).astype(ml_dtypes.bfloat16),
            "bw": bw.astype(ml_dtypes.bfloat16),
            "ident": ident,
        })
    return in_maps


def kernel(x, attn_bias, ln_gamma, ln_beta, w_qkv, w_out):
    in_maps = build_in_maps(x, attn_bias, ln_gamma, ln_beta, w_qkv, w_out)
    bw_zero = all(np.all(m["bw"] == 0) for m in in_maps)
    nc = _get_program(bw_zero)
    res = run_bass_kernel_spmd(nc, in_maps, core_ids=list(range(N_CORES)))
    out = np.zeros((B, N, D), dtype=np.float32)
    for c in range(N_CORES):
        out += res.results[c]["y"].astype(np.float32)
    return out
